# revision 1
# baseline (speedup 1.0000x reference)
"""Trainium2 Bass kernel for a post-norm decoder block (B=1, T=4096, C=768, 12 heads, MLP x4).

Sharding: strided data-parallel over the sequence. Core c owns tokens c::8
(512 tokens) -> every core has an identical causal-attention workload.

Pipeline: the Q/K/V projections are computed per head pair (128 features),
and each head pair's K^T + V payload is AllGathered in its own small
collective. The six collectives chain on the collective engine while
attention consumes earlier head pairs, so gather time hides behind the
softmax (ACT-engine exp) bound instead of serializing in front of it.

Layout: activations stay feature-major (x^T) for Q/K and W1; attention
logits come out transposed ([k, q]) which feeds A@V directly. Wo and W2
outputs are produced token-major (stationary activation slices) so both
LayerNorms run without PE transposes; only LN1->W1 transposes h (bf16).
Softmax denominators come from a ones-column appended to V. Head pairs
run QK at PE partition offsets 0/64 (concurrent row groups). Per-pair
PSUM accumulators are double-buffered and the softmax normalization
(reciprocal split into quarters + partition broadcast + multiply) is
emitted interleaved into the next pair's loop so it never blocks the
engine queues.
"""

import numpy as np
import ml_dtypes

import concourse.bass as bass
import concourse.mybir as mybir
import concourse.tile as tile
from concourse import bacc
from concourse.bass_utils import run_bass_kernel_spmd

f32 = mybir.dt.float32
bf16 = mybir.dt.bfloat16

NCORES = 8
T = 4096
C = 768
F = 3072
NH = 12
D = 64
TL = T // NCORES          # 512 local tokens per core
CCH = C // 128            # 6
FCH = F // 128            # 24
NQC = TL // 128           # 4 query chunks of 128
NSLOT = 4                 # kv slots: 128 local columns each
NHP = NH // 2             # 6 head pairs
EPS = 1e-5
K_HP = 128 * TL           # 65536 elems: one head pair's K^T payload
V_HP = NQC * 128 * 130    # 66560 elems: one head pair's V (+ones) payload
K_ALL = NHP * K_HP        # 393216
V_ALL = NHP * V_HP        # 399360
KV_RANK = K_ALL + V_ALL   # combined per-rank K+V payload (one AllGather)
SCALE = 1.0 / np.sqrt(D)
ADD = mybir.AluOpType.add
MULT = mybir.AluOpType.mult
SUB = mybir.AluOpType.subtract


def _ap(handle, offset, pattern):
    return bass.AP(tensor=handle, offset=offset, ap=[list(p) for p in pattern])


def build_nc():
    nc = bacc.Bacc("TRN2", target_bir_lowering=False, debug=False, num_devices=NCORES)

    # ---- I/O ----
    xT_in = nc.declare_dram_parameter("xT", [C, TL], bf16, isOutput=False)
    xtk_in = nc.declare_dram_parameter("xtk", [TL, C], f32, isOutput=False)
    mk_in = nc.declare_dram_parameter("masks", [128, NCORES * 128], bf16, isOutput=False)
    wq_in = nc.declare_dram_parameter("wq", [C, C], bf16, isOutput=False)
    wk_in = nc.declare_dram_parameter("wk", [C, C], bf16, isOutput=False)
    wv_in = nc.declare_dram_parameter("wv", [C, C], bf16, isOutput=False)
    wo_in = nc.declare_dram_parameter("wo", [C, C], bf16, isOutput=False)
    w1_in = nc.declare_dram_parameter("w1", [C, F], bf16, isOutput=False)
    w2_in = nc.declare_dram_parameter("w2", [F, C], bf16, isOutput=False)
    # packed per-partition (feature-major) biases: bq(6) bk(6) b1(24)
    colb_in = nc.declare_dram_parameter("colb", [128, 36], f32, isOutput=False)
    # packed broadcast-row consts: bo bv g1 h1 b2 g2 h2
    rowb_in = nc.declare_dram_parameter("rowb", [7, C], f32, isOutput=False)
    y_out = nc.declare_dram_parameter("y", [TL, C], f32, isOutput=True)

    k_loc = nc.dram_tensor("k_loc", [K_ALL], bf16)
    k_gath = nc.dram_tensor("k_gath", [NCORES * K_ALL], bf16, addr_space="Shared")
    v_loc = nc.dram_tensor("v_loc", [V_ALL], bf16)
    v_gath = nc.dram_tensor("v_gath", [NCORES * V_ALL], bf16, addr_space="Shared")

    id_bf_d = nc.inline_tensor(np.eye(128).astype(ml_dtypes.bfloat16), name="id_bf_d")

    with tile.TileContext(nc) as tc:
        import contextlib
        with contextlib.ExitStack() as ctx:
            consts = ctx.enter_context(tc.tile_pool(name="consts", bufs=1))
            actp = ctx.enter_context(tc.tile_pool(name="actp", bufs=1))
            w1pool = ctx.enter_context(tc.tile_pool(name="w1pool", bufs=1))
            postp = ctx.enter_context(tc.tile_pool(name="postp", bufs=1))

            eps_t = consts.tile([128, 1], f32, name="eps_t")
            nc.vector.memset(eps_t, EPS)
            colb = consts.tile([128, 36], f32, name="colb")
            nc.sync.dma_start(out=colb, in_=colb_in[:, :])
            bq_sb = colb[:, 0:CCH]
            bk_sb = colb[:, CCH:2 * CCH]
            b1_sb = colb[:, 2 * CCH:2 * CCH + FCH]

            # warm the ACT exp table before attention needs it
            dume = consts.tile([1, 1], f32, name="dume")
            nc.scalar.activation(out=dume, in_=eps_t[0:1, 0:1],
                                 func=mybir.ActivationFunctionType.Exp, scale=1.0)

            aoT = [actp.tile([128, TL], bf16, name=f"aoT_{i}") for i in range(CCH)]
            bc = {}

            def bc_load(nm):
                j = ["bo", "bv", "g1", "h1", "b2", "g2", "h2"].index(nm)
                t = consts.tile([128, C], f32, name=f"bc_{nm}")
                nc.sync.dma_start(out=t, in_=_ap(rowb_in, j * C, [[0, 128], [1, C]]))
                bc[nm] = t

            with tc.tile_pool(name="qkvlive", bufs=1) as qkvlive:
                qt_b = [qkvlive.tile([128, TL], bf16, name=f"qt_{ch}")
                        for ch in range(CCH)]

                # ---- per-head-pair K/Q/V + chained AllGathers ----
                with tc.tile_pool(name="wproj", bufs=2) as wproj, \
                     tc.tile_pool(name="psumq", bufs=4, space="PSUM") as psum:
                    # critical-path loads first: x^T (bf16), then weights
                    xt_b = []
                    for ch in range(CCH):
                        tb = wproj.tile([128, TL], bf16, name=f"xt_b_{ch}")
                        nc.sync.dma_start(
                            out=tb, in_=xT_in[128 * ch:128 * (ch + 1), :])
                        xt_b.append(tb)

                    def load_ws(handle, pfx):
                        ts = []
                        for kch in range(CCH):
                            t = wproj.tile([128, C], bf16, name=f"{pfx}_{kch}")
                            nc.sync.dma_start(
                                out=t, in_=handle[128 * kch:128 * (kch + 1), :])
                            ts.append(t)
                        return ts

                    wk_t = load_ws(wk_in, "wk")
                    wv_t = load_ws(wv_in, "wv")
                    wq_t = load_ws(wq_in, "wq")
                    bc_load("bv")

                    # ---- K^T for all head pairs ----
                    for hp in range(NHP):
                        lo = 128 * hp
                        ps = psum.tile([128, TL], f32, name="pp", tag="pp")
                        for kch in range(CCH):
                            nc.tensor.matmul(
                                ps, lhsT=wk_t[kch][:, lo:lo + 128],
                                rhs=xt_b[kch], start=(kch == 0), stop=(kch == CCH - 1))
                        kt = wproj.tile([128, TL], bf16, name=f"kt_{hp}",
                                        tag="kt_t", bufs=2)
                        nc.vector.tensor_scalar(
                            out=kt, in0=ps, scalar1=bk_sb[:, hp:hp + 1],
                            scalar2=None, op0=ADD)
                        nc.sync.dma_start(
                            out=_ap(k_loc, hp * K_HP, [[TL, 128], [1, TL]]),
                            in_=kt)
                    nc.gpsimd.collective_compute(
                        "AllGather", mybir.AluOpType.bypass,
                        replica_groups=[list(range(NCORES))],
                        ins=[k_loc[:]], outs=[k_gath[:]])

                    # ---- V (token-major, hp-major dram layout) ----
                    for tch in range(NQC):
                        for nh2 in range(2):
                            ps = psum.tile([128, 384], f32, name="pv", tag="pv")
                            for kch in range(CCH):
                                nc.tensor.matmul(
                                    ps,
                                    lhsT=xt_b[kch][:, 128 * tch:128 * (tch + 1)],
                                    rhs=wv_t[kch][:, 384 * nh2:384 * (nh2 + 1)],
                                    start=(kch == 0), stop=(kch == CCH - 1))
                            vt = wproj.tile([128, 6, 65], bf16,
                                            name=f"v_{tch}_{nh2}", tag="v_t", bufs=3)
                            nc.vector.tensor_tensor(
                                out=vt[:, :, 0:D],
                                in0=ps.rearrange("p (h d) -> p h d", d=D),
                                in1=bc["bv"][:, 384 * nh2:384 * (nh2 + 1)].rearrange(
                                    "p (h d) -> p h d", d=D),
                                op=ADD)
                            nc.vector.memset(vt[:, :, D:D + 1], 1.0)
                            nc.sync.dma_start(
                                out=_ap(v_loc,
                                        3 * nh2 * V_HP + tch * 130,
                                        [[NQC * 130, 128], [V_HP, 3], [1, 130]]),
                                in_=vt)

                    nc.gpsimd.collective_compute(
                        "AllGather", mybir.AluOpType.bypass,
                        replica_groups=[list(range(NCORES))],
                        ins=[v_loc[:]], outs=[v_gath[:]])

                    # masks: needed at attention start, small
                    mk_sb = consts.tile([128, NCORES * 128], bf16, name="mk_sb")
                    nc.sync.dma_start(out=mk_sb, in_=mk_in[:, :])

                    # ---- Q^T (overlaps the collective) ----
                    for hp in range(NHP):
                        lo = 128 * hp
                        ps = psum.tile([128, TL], f32, name="pp", tag="pp")
                        for kch in range(CCH):
                            nc.tensor.matmul(
                                ps, lhsT=wq_t[kch][:, lo:lo + 128],
                                rhs=xt_b[kch], start=(kch == 0), stop=(kch == CCH - 1))
                        nc.vector.tensor_scalar(
                            out=qt_b[hp], in0=ps, scalar1=bq_sb[:, hp:hp + 1],
                            scalar2=None, op0=ADD)

                # ---- attention ----
                with tc.tile_pool(name="kvstage", bufs=1) as kvstage, \
                     tc.tile_pool(name="atw", bufs=4) as atw, \
                     tc.tile_pool(name="atp", bufs=1, space="PSUM") as atp:
                    pending = []   # deferred normalize pieces from previous hp

                    def emit_pending(n):
                        for _ in range(min(n, len(pending))):
                            pending.pop(0)()

                    for hp in range(NHP):
                        ktg = kvstage.tile([128, NCORES, TL], bf16, name=f"ktg_{hp}",
                                           tag="ktg", bufs=2)
                        nc.sync.dma_start(
                            out=ktg,
                            in_=_ap(k_gath, hp * K_HP,
                                    [[TL, 128], [K_ALL, NCORES], [1, TL]]))
                        vag = kvstage.tile([128, NCORES, NQC, 130], bf16,
                                           name=f"vag_{hp}", tag="vag", bufs=2)
                        nc.sync.dma_start(
                            out=vag,
                            in_=_ap(v_gath, hp * V_HP,
                                    [[NQC * 130, 128], [V_ALL, NCORES],
                                     [1, NQC * 130]]))

                        acc = atp.tile([65, 2, TL], f32, name="acc", tag="acc",
                                       bufs=2)
                        first = True
                        it = 0
                        for s in range(NSLOT):
                            q0 = 128 * s
                            nq = TL - q0
                            for r in range(NCORES):
                                lg = atp.tile([128, 2, TL], f32, name="lg",
                                              tag="lg", bufs=2)
                                for i in range(2):
                                    ho = 64 * i
                                    nc.tensor.matmul(
                                        lg[:, i, 0:nq],
                                        lhsT=ktg[ho:ho + 64, r,
                                                 128 * s:128 * (s + 1)],
                                        rhs=qt_b[hp][ho:ho + 64, q0:TL],
                                        start=True, stop=True)
                                pr = atw.tile([128, 2, TL], bf16, name="pr",
                                              tag="pr", bufs=8)
                                nc.scalar.activation(
                                    out=pr[:, :, 0:nq], in_=lg[:, :, 0:nq],
                                    func=mybir.ActivationFunctionType.Exp,
                                    scale=SCALE)
                                mb = mk_sb[:, 128 * r:128 * (r + 1)]
                                nc.vector.tensor_tensor(
                                    out=pr[:, :, 0:128], in0=pr[:, :, 0:128],
                                    in1=bass.AP(tensor=mb.tensor, offset=mb.offset,
                                                ap=[list(mb.ap[0]), [0, 2],
                                                    list(mb.ap[1])]),
                                    op=MULT)
                                for i in range(2):
                                    nc.tensor.matmul(
                                        acc[:, i, q0:TL],
                                        lhsT=vag[:, r, s, 65 * i:65 * (i + 1)],
                                        rhs=pr[:, i, 0:nq],
                                        start=first,
                                        stop=(s == NSLOT - 1 and r == NCORES - 1))
                                first = False
                                it += 1
                                if it % 3 == 0:
                                    emit_pending(1)

                        # deferred normalize: quarters of the reciprocal, then
                        # broadcast + multiply per head, interleaved into the
                        # next head pair's loop
                        def make_norm(hp, acc):
                            rec = atw.tile([1, 2, TL], f32, name="rec", tag="rec",
                                           bufs=2)
                            pieces = []
                            for i in range(2):
                                for j in range(2):
                                    def recp(i=i, j=j, rec=rec, acc=acc):
                                        nc.vector.reciprocal(
                                            out=rec[:, i, 256 * j:256 * (j + 1)],
                                            in_=acc[D:D + 1, i,
                                                    256 * j:256 * (j + 1)])
                                    pieces.append(recp)

                                def norm(i=i, rec=rec, acc=acc, hp=hp):
                                    brd = atw.tile([64, TL], f32, name="brd",
                                                   tag=f"brd{i}", bufs=2)
                                    nc.gpsimd.partition_broadcast(brd, rec[:, i, :])
                                    nc.vector.tensor_tensor(
                                        out=aoT[hp][64 * i:64 * (i + 1), :],
                                        in0=acc[0:D, i, :], in1=brd, op=MULT)
                                pieces.append(norm)
                            return pieces

                        emit_pending(99)
                        pending = make_norm(hp, acc)
                    emit_pending(99)

            # ---- post-attention loads: issued behind the attention staging
            # DMAs so they don't contend with the collective sessions ----
            for nm in ["bo", "g1", "h1", "b2", "g2", "h2"]:
                bc_load(nm)
            id_bf = consts.tile([128, 128], bf16, name="id_bf")
            nc.sync.dma_start(out=id_bf, in_=id_bf_d[:])
            wo_t = []
            for kch in range(CCH):
                t = w1pool.tile([128, C], bf16, name=f"wo_{kch}")
                nc.sync.dma_start(out=t, in_=wo_in[128 * kch:128 * (kch + 1), :])
                wo_t.append(t)
            w1_t = []
            for kch in range(CCH):
                t = w1pool.tile([128, F], bf16, name=f"w1_{kch}")
                nc.sync.dma_start(out=t, in_=w1_in[128 * kch:128 * (kch + 1), :])
                w1_t.append(t)
            x_tok = []
            for tch in range(NQC):
                t = postp.tile([128, C], f32, name=f"xtok_{tch}")
                nc.sync.dma_start(out=t, in_=xtk_in[128 * tch:128 * (tch + 1), :])
                x_tok.append(t)

            # ---- prefetch W2 now that the KV staging SBUF is free ----
            w2pool = ctx.enter_context(tc.tile_pool(name="w2pool", bufs=1))
            w2_t = []
            for kch in range(FCH):
                t = w2pool.tile([128, C], bf16, name=f"w2t_{kch}")
                nc.sync.dma_start(out=t, in_=w2_in[128 * kch:128 * (kch + 1), :])
                w2_t.append(t)

            h_tok = [postp.tile([128, C], bf16, name=f"htok_{t}") for t in range(NQC)]
            hT = [postp.tile([128, TL], bf16, name=f"hT_{i}") for i in range(CCH)]
            mup = [postp.tile([128, TL], bf16, name=f"mup_{i}") for i in range(FCH)]
            lnw = ctx.enter_context(tc.tile_pool(name="lnw", bufs=2))

            def layernorm_apply(r_tile, g_bc, h_bc, out_tile, tag):
                st = lnw.tile([128, 3, 6], f32, name=f"st{tag}", tag=f"st{tag}")
                for sg in range(3):
                    nc.vector.bn_stats(
                        out=st[:, sg, :], in_=r_tile[:, 256 * sg:256 * (sg + 1)])
                mv = lnw.tile([128, 2], f32, name=f"mv{tag}", tag=f"mv{tag}")
                nc.vector.bn_aggr(out=mv, in_=st)
                sd = lnw.tile([128, 1], f32, name=f"sd{tag}", tag=f"sd{tag}")
                nc.scalar.activation(
                    out=sd, in_=mv[:, 1:2],
                    func=mybir.ActivationFunctionType.Sqrt, bias=eps_t, scale=1.0)
                rs = lnw.tile([128, 1], f32, name=f"rs{tag}", tag=f"rs{tag}")
                nc.vector.reciprocal(out=rs, in_=sd)
                tn = lnw.tile([128, C], f32, name=f"tn{tag}", tag=f"tn{tag}")
                nc.vector.tensor_scalar(
                    out=tn, in0=r_tile, scalar1=mv[:, 0:1], scalar2=rs,
                    op0=SUB, op1=MULT)
                nc.vector.tensor_tensor(out=tn, in0=tn, in1=g_bc, op=MULT)
                nc.vector.tensor_tensor(out=out_tile, in0=tn, in1=h_bc, op=ADD)

            # ---- Wo projection token-major + residual + LN1 + h transpose ----
            with tc.tile_pool(name="wop", bufs=1, space="PSUM") as wop:
                for tch in range(NQC):
                    psA = wop.tile([128, 384], f32, name="woA", tag="woA", bufs=2)
                    psB = wop.tile([128, 384], f32, name="woB", tag="woB", bufs=2)
                    for kch in range(CCH):
                        lt = aoT[kch][:, 128 * tch:128 * (tch + 1)]
                        nc.tensor.matmul(psA, lhsT=lt, rhs=wo_t[kch][:, 0:384],
                                         start=(kch == 0), stop=(kch == CCH - 1))
                        nc.tensor.matmul(psB, lhsT=lt, rhs=wo_t[kch][:, 384:C],
                                         start=(kch == 0), stop=(kch == CCH - 1))
                    r1 = lnw.tile([128, C], f32, name="r1", tag="r1")
                    nc.vector.tensor_tensor(out=r1[:, 0:384], in0=psA,
                                            in1=bc["bo"][:, 0:384], op=ADD)
                    nc.vector.tensor_tensor(out=r1[:, 384:C], in0=psB,
                                            in1=bc["bo"][:, 384:C], op=ADD)
                    nc.vector.tensor_tensor(out=r1, in0=r1, in1=x_tok[tch], op=ADD)
                    layernorm_apply(r1, bc["g1"], bc["h1"], h_tok[tch], "1")
                    for ch in range(CCH):
                        tp = wop.tile([128, 128], bf16, name="tp", tag="tp", bufs=2)
                        nc.tensor.transpose(
                            tp, in_=h_tok[tch][:, 128 * ch:128 * (ch + 1)],
                            identity=id_bf)
                        nc.scalar.copy(
                            out=hT[ch][:, 128 * tch:128 * (tch + 1)], in_=tp)

            # ---- fused W1+gelu and W2 (token-major), pipelined over F-chunks ----
            # PSUM: pp1 (2 banks) + psC/psD for two token chunks (4 banks) = 6.
            # Token chunks are processed in two passes; W1 runs in the first.
            with tc.tile_pool(name="mlpp", bufs=1, space="PSUM") as mlpp:

                def w2_pass(tchs, with_w1):
                    psC = {t: mlpp.tile([128, 384], f32, name=f"w2A{t}",
                                        tag=f"w2A_{t % 2}", bufs=1) for t in tchs}
                    psD = {t: mlpp.tile([128, 384], f32, name=f"w2B{t}",
                                        tag=f"w2B_{t % 2}", bufs=1) for t in tchs}

                    def w2_chunk(m):
                        for tch in tchs:
                            lt = mup[m][:, 128 * tch:128 * (tch + 1)]
                            nc.tensor.matmul(
                                psC[tch], lhsT=lt, rhs=w2_t[m][:, 0:384],
                                start=(m == 0), stop=(m == FCH - 1))
                            nc.tensor.matmul(
                                psD[tch], lhsT=lt, rhs=w2_t[m][:, 384:C],
                                start=(m == 0), stop=(m == FCH - 1))

                    if with_w1:
                        for m in range(FCH):
                            ps = mlpp.tile([128, TL], f32, name="pp1", tag="pp1",
                                           bufs=2)
                            for kch in range(CCH):
                                nc.tensor.matmul(
                                    ps, lhsT=w1_t[kch][:, 128 * m:128 * (m + 1)],
                                    rhs=hT[kch],
                                    start=(kch == 0), stop=(kch == CCH - 1))
                            nc.scalar.activation(
                                out=mup[m], in_=ps,
                                func=mybir.ActivationFunctionType.Gelu,
                                bias=b1_sb[:, m:m + 1], scale=1.0)
                            if m > 0:
                                w2_chunk(m - 1)
                        w2_chunk(FCH - 1)
                    else:
                        for m in range(FCH):
                            w2_chunk(m)

                    # residual + LN2 -> y for these token chunks
                    for tch in tchs:
                        r2 = lnw.tile([128, C], f32, name="r2", tag="r2")
                        nc.vector.tensor_tensor(out=r2[:, 0:384], in0=psC[tch],
                                                in1=bc["b2"][:, 0:384], op=ADD)
                        nc.vector.tensor_tensor(out=r2[:, 384:C], in0=psD[tch],
                                                in1=bc["b2"][:, 384:C], op=ADD)
                        nc.vector.tensor_tensor(out=r2, in0=r2, in1=h_tok[tch],
                                                op=ADD)
                        yt = lnw.tile([128, C], f32, name="yt", tag="yt")
                        layernorm_apply(r2, bc["g2"], bc["h2"], yt, "2")
                        nc.sync.dma_start(
                            out=y_out[128 * tch:128 * (tch + 1), :], in_=yt)

                w2_pass([0, 1], True)
                w2_pass([2, 3], False)

    nc.compile()
    return nc


_NC_CACHE = None


def _get_nc():
    global _NC_CACHE
    if _NC_CACHE is None:
        _NC_CACHE = build_nc()
    return _NC_CACHE


def make_in_maps(inputs):
    x = np.asarray(inputs["x"], dtype=np.float32)      # [1, T, C]
    to_bf = lambda a: np.asarray(a, dtype=np.float32).astype(ml_dtypes.bfloat16)
    f32a = lambda k: np.asarray(inputs[k], np.float32)
    colb = np.concatenate([
        f32a("bq").reshape(CCH, 128).T,
        f32a("bk").reshape(CCH, 128).T,
        f32a("b1").reshape(FCH, 128).T,
    ], axis=1)                                          # [128, 36]
    rowb = np.stack([f32a("bo"), f32a("bv"), f32a("ln1_g"), f32a("ln1_b"),
                     f32a("b2"), f32a("ln2_g"), f32a("ln2_b")])  # [7, 768]
    shared = {
        "wq": to_bf(inputs["Wq"]), "wk": to_bf(inputs["Wk"]),
        "wv": to_bf(inputs["Wv"]), "wo": to_bf(inputs["Wo"]),
        "w1": to_bf(inputs["W1"]), "w2": to_bf(inputs["W2"]),
        "colb": np.ascontiguousarray(colb), "rowb": np.ascontiguousarray(rowb),
    }
    ki = np.arange(128)[:, None]
    qi = np.arange(128)[None, :]
    in_maps = []
    for c in range(NCORES):
        xs = x[0, c::NCORES, :]                                 # [TL, C]
        # multiplicative 0/1 causal masks for the diagonal kv slot, packed
        masks = np.concatenate([
            (NCORES * ki + r <= NCORES * qi + c) for r in range(NCORES)
        ], axis=1).astype(ml_dtypes.bfloat16)                   # [128, 8*128]
        m = dict(shared)
        m["xT"] = np.ascontiguousarray(xs.T).astype(ml_dtypes.bfloat16)
        m["xtk"] = np.ascontiguousarray(xs)
        m["masks"] = np.ascontiguousarray(masks)
        in_maps.append(m)
    return in_maps


def kernel(**inputs):
    nc = _get_nc()
    in_maps = make_in_maps(inputs)
    res = run_bass_kernel_spmd(nc, in_maps, list(range(NCORES)))
    x = np.asarray(inputs["x"])
    out = np.empty((1, T, C), dtype=np.float32)
    for c in range(NCORES):
        out[0, c::NCORES, :] = res.results[c]["y"]
    return out.astype(x.dtype) if x.dtype != np.float32 else out



# revision 9
# speedup vs baseline: 1.0842x; 1.0842x over previous
"""Trainium2 Bass kernel for a post-norm decoder block (B=1, T=4096, C=768, 12 heads, MLP x4).

Sharding: strided data-parallel over the sequence. Core c owns tokens c::8
(512 tokens) -> every core has an identical causal-attention workload.

Pipeline: Q/K/V projections are computed per head pair (128 features);
K^T and V for all head pairs go out in ONE combined AllGather (halves the
collective-chain latency vs separate K and V gathers and lets attention
start with both operands in hand). Dummy matmul bursts at kernel start
and across the gather window keep the PE HAM un-throttled (K=8/8).

Layout: activations stay feature-major (x^T) for Q/K and W1; attention
logits come out transposed ([k, q]) which feeds A@V directly. Wo and W2
outputs are produced token-major (stationary activation slices) so both
LayerNorms run without PE transposes; only LN1->W1 transposes h (bf16).
Softmax denominators come from a ones-column appended to V; their
reciprocal runs on the Scalar engine as exp(-ln(x)) (both functions live
in the natural_log_exp table set, so no table switching) instead of the
slow single-partition DVE reciprocal, keeping the Vector engine free for
the mask multiplies that gate A@V. LayerNorm rsqrt uses the same ln/exp
pair. Head pairs run QK at PE partition offsets 0/64 (concurrent row
groups). Per-pair PSUM accumulators are double-buffered and the softmax
normalization is emitted interleaved into the next pair's loop.
Post-attention weights (Wo/W1/W2/x residual) prefetch during the gather.
"""

import numpy as np
import ml_dtypes

import concourse.bass as bass
import concourse.mybir as mybir
import concourse.tile as tile
from concourse import bacc
from concourse.bass_utils import run_bass_kernel_spmd

f32 = mybir.dt.float32
bf16 = mybir.dt.bfloat16

NCORES = 8
T = 4096
C = 768
F = 3072
NH = 12
D = 64
TL = T // NCORES          # 512 local tokens per core
CCH = C // 128            # 6
FCH = F // 128            # 24
NQC = TL // 128           # 4 query chunks of 128
NSLOT = 4                 # kv slots: 128 local columns each
NHP = NH // 2             # 6 head pairs
EPS = 1e-5
K_HP = 128 * TL           # 65536 elems: one head pair's K^T payload
V_HP = NQC * 128 * 130    # 66560 elems: one head pair's V (+ones) payload
K_ALL = NHP * K_HP        # 393216
V_ALL = NHP * V_HP        # 399360
KV_RANK = K_ALL + V_ALL   # combined per-rank K+V payload (one AllGather)
SCALE = 1.0 / np.sqrt(D)
ADD = mybir.AluOpType.add
MULT = mybir.AluOpType.mult
SUB = mybir.AluOpType.subtract


def _ap(handle, offset, pattern):
    return bass.AP(tensor=handle, offset=offset, ap=[list(p) for p in pattern])


class _Bacc(bacc.Bacc):
    """Bacc whose activation-table placement maps Exp and Ln exclusively to
    the natural_log_exp_and_others set, so the softmax exp, the exp(-ln(x))
    denominators and the LayerNorm rsqrt all share one resident table
    (the default first-match policy alternates natural_log / exp_and_others
    and reloads tables on every switch)."""

    def insert_act_table_loads(self):
        import bass_rust as _bass_rust
        from concourse.hw_specs import get_activation_tables
        has_activation = any(
            isinstance(i, mybir.InstActivation)
            for b in self.main_func.blocks
            for i in b.instructions
        )
        if not has_activation:
            return
        exp_ln = {mybir.ActivationFunctionType.Exp,
                  mybir.ActivationFunctionType.Ln}
        tables = []
        for name, fns in get_activation_tables(self.m.arch).items():
            if name != "natural_log_exp_and_others":
                fns = set(fns) - exp_ln
            tables.append((name, set(fns)))
        _bass_rust.insert_act_table_loads(self, tables)


def build_nc():
    nc = _Bacc("TRN2", target_bir_lowering=False, debug=False, num_devices=NCORES)

    # ---- I/O ----
    xT_in = nc.declare_dram_parameter("xT", [C, TL], bf16, isOutput=False)
    xtk_in = nc.declare_dram_parameter("xtk", [TL, C], f32, isOutput=False)
    mk_in = nc.declare_dram_parameter("masks", [128, NCORES * 128], bf16, isOutput=False)
    wq_in = nc.declare_dram_parameter("wq", [C, C], bf16, isOutput=False)
    wk_in = nc.declare_dram_parameter("wk", [C, C], bf16, isOutput=False)
    wv_in = nc.declare_dram_parameter("wv", [C, C], bf16, isOutput=False)
    wo_in = nc.declare_dram_parameter("wo", [C, C], bf16, isOutput=False)
    w1_in = nc.declare_dram_parameter("w1", [C, F], bf16, isOutput=False)
    w2_in = nc.declare_dram_parameter("w2", [F, C], bf16, isOutput=False)
    # packed per-partition (feature-major) biases: bq(6) bk(6) b1(24)
    colb_in = nc.declare_dram_parameter("colb", [128, 36], f32, isOutput=False)
    # packed broadcast-row consts: bo bv g1 h1 b2 g2 h2
    rowb_in = nc.declare_dram_parameter("rowb", [7, C], f32, isOutput=False)
    y_out = nc.declare_dram_parameter("y", [TL, C], f32, isOutput=True)

    kv_loc = nc.dram_tensor("kv_loc", [KV_RANK], bf16)
    kv_gath = nc.dram_tensor("kv_gath", [NCORES * KV_RANK], bf16,
                             addr_space="Shared")

    id_bf_d = nc.inline_tensor(np.eye(128).astype(ml_dtypes.bfloat16), name="id_bf_d")

    with tile.TileContext(nc) as tc:
        import contextlib
        with contextlib.ExitStack() as ctx:
            consts = ctx.enter_context(tc.tile_pool(name="consts", bufs=1))
            actp = ctx.enter_context(tc.tile_pool(name="actp", bufs=1))
            w1pool = ctx.enter_context(tc.tile_pool(name="w1pool", bufs=1))
            postp = ctx.enter_context(tc.tile_pool(name="postp", bufs=1))
            warmp = ctx.enter_context(tc.tile_pool(name="warmp", bufs=1))

            # ---- PE warm-up: flip HAM to K=8/8 while the first DMAs land ----
            warm_sb = warmp.tile([128, 512], bf16, name="warm_sb")
            nc.vector.memset(warm_sb, 0.01)

            def pe_burn(n, tag):
                with tc.tile_pool(name=f"warmps{tag}", bufs=1,
                                  space="PSUM") as wpool:
                    wps = wpool.tile([128, 512], f32, name=f"wps{tag}")
                    for _ in range(n):
                        nc.tensor.matmul(wps, lhsT=warm_sb[:, 0:128],
                                         rhs=warm_sb, start=True, stop=True)

            pe_burn(40, "a")

            eps_t = consts.tile([128, 1], f32, name="eps_t")
            nc.vector.memset(eps_t, EPS)
            colb = consts.tile([128, 36], f32, name="colb")
            nc.sync.dma_start(out=colb, in_=colb_in[:, :])
            bq_sb = colb[:, 0:CCH]
            bk_sb = colb[:, CCH:2 * CCH]
            b1_sb = colb[:, 2 * CCH:2 * CCH + FCH]

            # warm the ACT ln/exp table before attention needs it
            dume = consts.tile([1, 1], f32, name="dume")
            nc.scalar.activation(out=dume, in_=eps_t[0:1, 0:1],
                                 func=mybir.ActivationFunctionType.Ln, scale=1.0)
            nc.scalar.activation(out=dume, in_=dume,
                                 func=mybir.ActivationFunctionType.Exp, scale=1.0)

            aoT = [actp.tile([128, TL], bf16, name=f"aoT_{i}") for i in range(CCH)]
            bc = {}

            def bc_load(nm):
                j = ["bo", "bv", "g1", "h1", "b2", "g2", "h2"].index(nm)
                t = consts.tile([128, C], f32, name=f"bc_{nm}")
                nc.sync.dma_start(out=t, in_=_ap(rowb_in, j * C, [[0, 128], [1, C]]))
                bc[nm] = t

            with tc.tile_pool(name="qkvlive", bufs=1) as qkvlive:
                qt_b = [qkvlive.tile([128, TL], bf16, name=f"qt_{ch}")
                        for ch in range(CCH)]

                # ---- per-head-pair K/Q/V + one combined AllGather ----
                with tc.tile_pool(name="wproj", bufs=2) as wproj, \
                     tc.tile_pool(name="psumq", bufs=4, space="PSUM") as psum:
                    # critical-path loads first: x^T (bf16), then weights
                    xt_b = []
                    for ch in range(CCH):
                        tb = wproj.tile([128, TL], bf16, name=f"xt_b_{ch}")
                        nc.sync.dma_start(
                            out=tb, in_=xT_in[128 * ch:128 * (ch + 1), :])
                        xt_b.append(tb)

                    def load_ws(handle, pfx):
                        ts = []
                        for kch in range(CCH):
                            t = wproj.tile([128, C], bf16, name=f"{pfx}_{kch}")
                            nc.sync.dma_start(
                                out=t, in_=handle[128 * kch:128 * (kch + 1), :])
                            ts.append(t)
                        return ts

                    wk_t = load_ws(wk_in, "wk")
                    wv_t = load_ws(wv_in, "wv")
                    wq_t = load_ws(wq_in, "wq")
                    bc_load("bv")

                    # ---- K^T for all head pairs ----
                    for hp in range(NHP):
                        lo = 128 * hp
                        ps = psum.tile([128, TL], f32, name="pp", tag="pp")
                        for kch in range(CCH):
                            nc.tensor.matmul(
                                ps, lhsT=wk_t[kch][:, lo:lo + 128],
                                rhs=xt_b[kch], start=(kch == 0), stop=(kch == CCH - 1))
                        kt = wproj.tile([128, TL], bf16, name=f"kt_{hp}",
                                        tag="kt_t", bufs=2)
                        nc.vector.tensor_scalar(
                            out=kt, in0=ps, scalar1=bk_sb[:, hp:hp + 1],
                            scalar2=None, op0=ADD)
                        nc.sync.dma_start(
                            out=_ap(kv_loc, hp * K_HP, [[TL, 128], [1, TL]]),
                            in_=kt)

                    # ---- V (token-major, hp-major dram layout) ----
                    for tch in range(NQC):
                        for nh2 in range(2):
                            ps = psum.tile([128, 384], f32, name="pv", tag="pv")
                            for kch in range(CCH):
                                nc.tensor.matmul(
                                    ps,
                                    lhsT=xt_b[kch][:, 128 * tch:128 * (tch + 1)],
                                    rhs=wv_t[kch][:, 384 * nh2:384 * (nh2 + 1)],
                                    start=(kch == 0), stop=(kch == CCH - 1))
                            vt = wproj.tile([128, 6, 65], bf16,
                                            name=f"v_{tch}_{nh2}", tag="v_t", bufs=3)
                            nc.vector.tensor_tensor(
                                out=vt[:, :, 0:D],
                                in0=ps.rearrange("p (h d) -> p h d", d=D),
                                in1=bc["bv"][:, 384 * nh2:384 * (nh2 + 1)].rearrange(
                                    "p (h d) -> p h d", d=D),
                                op=ADD)
                            nc.vector.memset(vt[:, :, D:D + 1], 1.0)
                            nc.sync.dma_start(
                                out=_ap(kv_loc,
                                        K_ALL + 3 * nh2 * V_HP + tch * 130,
                                        [[NQC * 130, 128], [V_HP, 3], [1, 130]]),
                                in_=vt)

                    # one combined K+V AllGather
                    nc.gpsimd.collective_compute(
                        "AllGather", mybir.AluOpType.bypass,
                        replica_groups=[list(range(NCORES))],
                        ins=[kv_loc[:]], outs=[kv_gath[:]])

                    # masks: needed at attention start, small
                    mk_sb = consts.tile([128, NCORES * 128], bf16, name="mk_sb")
                    nc.sync.dma_start(out=mk_sb, in_=mk_in[:, :])

                    # ---- Q^T (overlaps the collective) ----
                    for hp in range(NHP):
                        lo = 128 * hp
                        ps = psum.tile([128, TL], f32, name="pp", tag="pp")
                        for kch in range(CCH):
                            nc.tensor.matmul(
                                ps, lhsT=wq_t[kch][:, lo:lo + 128],
                                rhs=xt_b[kch], start=(kch == 0), stop=(kch == CCH - 1))
                        nc.vector.tensor_scalar(
                            out=qt_b[hp], in0=ps, scalar1=bq_sb[:, hp:hp + 1],
                            scalar2=None, op0=ADD)

                # ---- Wo / residual prefetch: issued while the gather runs so
                # the post-attention phases start with data resident ----
                for nm in ["bo", "g1", "h1", "b2", "g2", "h2"]:
                    bc_load(nm)
                id_bf = consts.tile([128, 128], bf16, name="id_bf")
                nc.sync.dma_start(out=id_bf, in_=id_bf_d[:])
                wo_t = []
                for kch in range(CCH):
                    t = w1pool.tile([128, C], bf16, name=f"wo_{kch}")
                    nc.sync.dma_start(out=t, in_=wo_in[128 * kch:128 * (kch + 1), :])
                    wo_t.append(t)
                x_tok = []
                for tch in range(NQC):
                    t = postp.tile([128, C], f32, name=f"xtok_{tch}")
                    nc.sync.dma_start(out=t, in_=xtk_in[128 * tch:128 * (tch + 1), :])
                    x_tok.append(t)
                w1_t = []

                # keep the PE array active across the gather window so
                # attention starts at full clock
                pe_burn(192, "b")

                # ---- attention ----
                with tc.tile_pool(name="kvstage", bufs=1) as kvstage, \
                     tc.tile_pool(name="atw", bufs=4) as atw, \
                     tc.tile_pool(name="atp", bufs=1, space="PSUM") as atp:
                    pending = []   # deferred normalize pieces from previous hp

                    def emit_pending(n):
                        for _ in range(min(n, len(pending))):
                            pending.pop(0)()

                    for hp in range(NHP):
                        if hp == NHP - 2:
                            # W1 prefetch: SBUF for it frees up once the early
                            # staging buffers retire; MLP needs it ~40us later
                            for kch in range(CCH):
                                t = w1pool.tile([128, F], bf16, name=f"w1_{kch}")
                                nc.sync.dma_start(
                                    out=t, in_=w1_in[128 * kch:128 * (kch + 1), :])
                                w1_t.append(t)
                        # per-rank staging so QK(r) starts as soon as rank r lands
                        ktg = kvstage.tile([128, NCORES, TL], bf16, name=f"ktg_{hp}",
                                           tag="ktg", bufs=2)
                        for r in range(NCORES):
                            nc.sync.dma_start(
                                out=ktg[:, r, :],
                                in_=_ap(kv_gath, r * KV_RANK + hp * K_HP,
                                        [[TL, 128], [1, TL]]))
                        vag = kvstage.tile([128, NCORES, NQC, 130], bf16,
                                           name=f"vag_{hp}", tag="vag", bufs=2)
                        for r in range(NCORES):
                            nc.sync.dma_start(
                                out=vag[:, r, :, :],
                                in_=_ap(kv_gath, r * KV_RANK + K_ALL + hp * V_HP,
                                        [[NQC * 130, 128], [1, NQC * 130]]))

                        acc = atp.tile([65, 2, TL], f32, name="acc", tag="acc",
                                       bufs=2)
                        first = True
                        it = 0
                        for s in range(NSLOT):
                            q0 = 128 * s
                            nq = TL - q0
                            for r in range(NCORES):
                                lg = atp.tile([128, 2, TL], f32, name="lg",
                                              tag="lg", bufs=2)
                                for i in range(2):
                                    ho = 64 * i
                                    nc.tensor.matmul(
                                        lg[:, i, 0:nq],
                                        lhsT=ktg[ho:ho + 64, r,
                                                 128 * s:128 * (s + 1)],
                                        rhs=qt_b[hp][ho:ho + 64, q0:TL],
                                        start=True, stop=True)
                                pr = atw.tile([128, 2, TL], bf16, name="pr",
                                              tag="pr", bufs=8)
                                nc.scalar.activation(
                                    out=pr[:, :, 0:nq], in_=lg[:, :, 0:nq],
                                    func=mybir.ActivationFunctionType.Exp,
                                    scale=SCALE)
                                mb = mk_sb[:, 128 * r:128 * (r + 1)]
                                nc.vector.tensor_tensor(
                                    out=pr[:, :, 0:128], in0=pr[:, :, 0:128],
                                    in1=bass.AP(tensor=mb.tensor, offset=mb.offset,
                                                ap=[list(mb.ap[0]), [0, 2],
                                                    list(mb.ap[1])]),
                                    op=MULT)
                                for i in range(2):
                                    nc.tensor.matmul(
                                        acc[:, i, q0:TL],
                                        lhsT=vag[:, r, s, 65 * i:65 * (i + 1)],
                                        rhs=pr[:, i, 0:nq],
                                        start=first,
                                        stop=(s == NSLOT - 1 and r == NCORES - 1))
                                first = False
                                it += 1
                                if it % 3 == 0:
                                    emit_pending(1)

                        # deferred normalize: reciprocal of the denominators on
                        # the Scalar engine as exp(-ln(x)) (same table set as
                        # the attention exp), then partition broadcast +
                        # multiply, interleaved into the next head pair's loop
                        def make_norm(hp, acc):
                            lnd = atw.tile([1, 2, TL], f32, name="lnd", tag="lnd",
                                           bufs=2)
                            rec = atw.tile([1, 2, TL], f32, name="rec", tag="rec",
                                           bufs=2)
                            pieces = []

                            def p_ln(lnd=lnd, acc=acc):
                                nc.scalar.activation(
                                    out=lnd, in_=acc[D:D + 1, :, :],
                                    func=mybir.ActivationFunctionType.Ln,
                                    scale=1.0)

                            def p_exp(rec=rec, lnd=lnd):
                                nc.scalar.activation(
                                    out=rec, in_=lnd,
                                    func=mybir.ActivationFunctionType.Exp,
                                    scale=-1.0)

                            pieces.append(p_ln)
                            pieces.append(p_exp)
                            for i in range(2):
                                def norm(i=i, rec=rec, acc=acc, hp=hp):
                                    brd = atw.tile([64, TL], f32, name="brd",
                                                   tag=f"brd{i}", bufs=2)
                                    nc.gpsimd.partition_broadcast(brd, rec[:, i, :])
                                    nc.vector.tensor_tensor(
                                        out=aoT[hp][64 * i:64 * (i + 1), :],
                                        in0=acc[0:D, i, :], in1=brd, op=MULT)
                                pieces.append(norm)
                            return pieces

                        emit_pending(99)
                        pending = make_norm(hp, acc)
                    emit_pending(99)

            # ---- W2 prefetch now that the KV staging SBUF is free ----
            w2pool = ctx.enter_context(tc.tile_pool(name="w2pool", bufs=1))
            w2_t = []
            for kch in range(FCH):
                t = w2pool.tile([128, C], bf16, name=f"w2t_{kch}")
                nc.sync.dma_start(out=t, in_=w2_in[128 * kch:128 * (kch + 1), :])
                w2_t.append(t)

            h_tok = [postp.tile([128, C], bf16, name=f"htok_{t}") for t in range(NQC)]
            hT = [postp.tile([128, TL], bf16, name=f"hT_{i}") for i in range(CCH)]
            mup = [postp.tile([128, TL], bf16, name=f"mup_{i}") for i in range(FCH)]
            lnw = ctx.enter_context(tc.tile_pool(name="lnw", bufs=2))

            def layernorm_apply(r_tile, g_bc, h_bc, out_tile, tag):
                st = lnw.tile([128, 3, 6], f32, name=f"st{tag}", tag=f"st{tag}")
                for sg in range(3):
                    nc.vector.bn_stats(
                        out=st[:, sg, :], in_=r_tile[:, 256 * sg:256 * (sg + 1)])
                mv = lnw.tile([128, 2], f32, name=f"mv{tag}", tag=f"mv{tag}")
                nc.vector.bn_aggr(out=mv, in_=st)
                # rsqrt(var + eps) = exp(-0.5 * ln(var + eps)): stays in the
                # natural_log_exp table set (no sqrt table switch)
                lnv = lnw.tile([128, 1], f32, name=f"lnv{tag}", tag=f"lnv{tag}")
                nc.scalar.activation(
                    out=lnv, in_=mv[:, 1:2],
                    func=mybir.ActivationFunctionType.Ln, bias=eps_t, scale=1.0)
                rs = lnw.tile([128, 1], f32, name=f"rs{tag}", tag=f"rs{tag}")
                nc.scalar.activation(
                    out=rs, in_=lnv,
                    func=mybir.ActivationFunctionType.Exp, scale=-0.5)
                tn = lnw.tile([128, C], f32, name=f"tn{tag}", tag=f"tn{tag}")
                nc.vector.tensor_scalar(
                    out=tn, in0=r_tile, scalar1=mv[:, 0:1], scalar2=rs,
                    op0=SUB, op1=MULT)
                nc.vector.tensor_tensor(out=tn, in0=tn, in1=g_bc, op=MULT)
                nc.vector.tensor_tensor(out=out_tile, in0=tn, in1=h_bc, op=ADD)

            # ---- Wo projection token-major + residual + LN1 + h transpose ----
            with tc.tile_pool(name="wop", bufs=1, space="PSUM") as wop:
                for tch in range(NQC):
                    psA = wop.tile([128, 384], f32, name="woA", tag="woA", bufs=2)
                    psB = wop.tile([128, 384], f32, name="woB", tag="woB", bufs=2)
                    for kch in range(CCH):
                        lt = aoT[kch][:, 128 * tch:128 * (tch + 1)]
                        nc.tensor.matmul(psA, lhsT=lt, rhs=wo_t[kch][:, 0:384],
                                         start=(kch == 0), stop=(kch == CCH - 1))
                        nc.tensor.matmul(psB, lhsT=lt, rhs=wo_t[kch][:, 384:C],
                                         start=(kch == 0), stop=(kch == CCH - 1))
                    r1 = lnw.tile([128, C], f32, name="r1", tag="r1")
                    nc.vector.tensor_tensor(out=r1[:, 0:384], in0=psA,
                                            in1=bc["bo"][:, 0:384], op=ADD)
                    nc.vector.tensor_tensor(out=r1[:, 384:C], in0=psB,
                                            in1=bc["bo"][:, 384:C], op=ADD)
                    nc.vector.tensor_tensor(out=r1, in0=r1, in1=x_tok[tch], op=ADD)
                    layernorm_apply(r1, bc["g1"], bc["h1"], h_tok[tch], "1")
                    for ch in range(CCH):
                        tp = wop.tile([128, 128], bf16, name="tp", tag="tp", bufs=2)
                        nc.tensor.transpose(
                            tp, in_=h_tok[tch][:, 128 * ch:128 * (ch + 1)],
                            identity=id_bf)
                        nc.scalar.copy(
                            out=hT[ch][:, 128 * tch:128 * (tch + 1)], in_=tp)

            # ---- fused W1+gelu and W2 (token-major), pipelined over F-chunks ----
            # PSUM: pp1 (2 banks) + psC/psD for two token chunks (4 banks) = 6.
            # Token chunks are processed in two passes; W1 runs in the first.
            with tc.tile_pool(name="mlpp", bufs=1, space="PSUM") as mlpp:

                def w2_pass(tchs, with_w1):
                    psC = {t: mlpp.tile([128, 384], f32, name=f"w2A{t}",
                                        tag=f"w2A_{t % 2}", bufs=1) for t in tchs}
                    psD = {t: mlpp.tile([128, 384], f32, name=f"w2B{t}",
                                        tag=f"w2B_{t % 2}", bufs=1) for t in tchs}

                    def w2_chunk(m):
                        for tch in tchs:
                            lt = mup[m][:, 128 * tch:128 * (tch + 1)]
                            nc.tensor.matmul(
                                psC[tch], lhsT=lt, rhs=w2_t[m][:, 0:384],
                                start=(m == 0), stop=(m == FCH - 1))
                            nc.tensor.matmul(
                                psD[tch], lhsT=lt, rhs=w2_t[m][:, 384:C],
                                start=(m == 0), stop=(m == FCH - 1))

                    if with_w1:
                        for m in range(FCH):
                            ps = mlpp.tile([128, TL], f32, name="pp1", tag="pp1",
                                           bufs=2)
                            for kch in range(CCH):
                                nc.tensor.matmul(
                                    ps, lhsT=w1_t[kch][:, 128 * m:128 * (m + 1)],
                                    rhs=hT[kch],
                                    start=(kch == 0), stop=(kch == CCH - 1))
                            nc.scalar.activation(
                                out=mup[m], in_=ps,
                                func=mybir.ActivationFunctionType.Gelu,
                                bias=b1_sb[:, m:m + 1], scale=1.0)
                            if m > 0:
                                w2_chunk(m - 1)
                        w2_chunk(FCH - 1)
                    else:
                        for m in range(FCH):
                            w2_chunk(m)

                    # residual + LN2 -> y for these token chunks
                    for tch in tchs:
                        r2 = lnw.tile([128, C], f32, name="r2", tag="r2")
                        nc.vector.tensor_tensor(out=r2[:, 0:384], in0=psC[tch],
                                                in1=bc["b2"][:, 0:384], op=ADD)
                        nc.vector.tensor_tensor(out=r2[:, 384:C], in0=psD[tch],
                                                in1=bc["b2"][:, 384:C], op=ADD)
                        nc.vector.tensor_tensor(out=r2, in0=r2, in1=h_tok[tch],
                                                op=ADD)
                        yt = lnw.tile([128, C], f32, name="yt", tag="yt")
                        layernorm_apply(r2, bc["g2"], bc["h2"], yt, "2")
                        nc.sync.dma_start(
                            out=y_out[128 * tch:128 * (tch + 1), :], in_=yt)

                w2_pass([0, 1], True)
                w2_pass([2, 3], False)

    nc.compile()
    return nc


_NC_CACHE = None


def _get_nc():
    global _NC_CACHE
    if _NC_CACHE is None:
        _NC_CACHE = build_nc()
    return _NC_CACHE


def make_in_maps(inputs):
    x = np.asarray(inputs["x"], dtype=np.float32)      # [1, T, C]
    to_bf = lambda a: np.asarray(a, dtype=np.float32).astype(ml_dtypes.bfloat16)
    f32a = lambda k: np.asarray(inputs[k], np.float32)
    colb = np.concatenate([
        f32a("bq").reshape(CCH, 128).T,
        f32a("bk").reshape(CCH, 128).T,
        f32a("b1").reshape(FCH, 128).T,
    ], axis=1)                                          # [128, 36]
    rowb = np.stack([f32a("bo"), f32a("bv"), f32a("ln1_g"), f32a("ln1_b"),
                     f32a("b2"), f32a("ln2_g"), f32a("ln2_b")])  # [7, 768]
    shared = {
        "wq": to_bf(inputs["Wq"]), "wk": to_bf(inputs["Wk"]),
        "wv": to_bf(inputs["Wv"]), "wo": to_bf(inputs["Wo"]),
        "w1": to_bf(inputs["W1"]), "w2": to_bf(inputs["W2"]),
        "colb": np.ascontiguousarray(colb), "rowb": np.ascontiguousarray(rowb),
    }
    ki = np.arange(128)[:, None]
    qi = np.arange(128)[None, :]
    in_maps = []
    for c in range(NCORES):
        xs = x[0, c::NCORES, :]                                 # [TL, C]
        # multiplicative 0/1 causal masks for the diagonal kv slot, packed
        masks = np.concatenate([
            (NCORES * ki + r <= NCORES * qi + c) for r in range(NCORES)
        ], axis=1).astype(ml_dtypes.bfloat16)                   # [128, 8*128]
        m = dict(shared)
        m["xT"] = np.ascontiguousarray(xs.T).astype(ml_dtypes.bfloat16)
        m["xtk"] = np.ascontiguousarray(xs)
        m["masks"] = np.ascontiguousarray(masks)
        in_maps.append(m)
    return in_maps


def kernel(**inputs):
    nc = _get_nc()
    in_maps = make_in_maps(inputs)
    res = run_bass_kernel_spmd(nc, in_maps, list(range(NCORES)))
    x = np.asarray(inputs["x"])
    out = np.empty((1, T, C), dtype=np.float32)
    for c in range(NCORES):
        out[0, c::NCORES, :] = res.results[c]["y"]
    return out.astype(x.dtype) if x.dtype != np.float32 else out


# revision 14
# speedup vs baseline: 1.0891x; 1.0045x over previous
"""Trainium2 Bass kernel for a post-norm decoder block (B=1, T=4096, C=768, 12 heads, MLP x4).

Sharding: strided data-parallel over the sequence. Core c owns tokens c::8
(512 tokens) -> every core has an identical causal-attention workload.

Pipeline: Q/K/V projections are computed per head pair (128 features);
K^T and V for all head pairs go out in ONE combined AllGather (halves the
collective-chain latency vs separate K and V gathers and lets attention
start with both operands in hand). Dummy matmul bursts at kernel start
and across the gather window keep the PE HAM un-throttled (K=8/8).

Layout: activations stay feature-major (x^T) for Q/K and W1; attention
logits come out transposed ([k, q]) which feeds A@V directly. Wo and W2
outputs are produced token-major (stationary activation slices) so both
LayerNorms run without PE transposes; only LN1->W1 transposes h (bf16).
Softmax denominators come from a ones-column appended to V; their
reciprocal runs on the Scalar engine as exp(-ln(x)) (both functions live
in the natural_log_exp table set, so no table switching) instead of the
slow single-partition DVE reciprocal, keeping the Vector engine free for
the mask multiplies that gate A@V. LayerNorm rsqrt uses the same ln/exp
pair. Head pairs run QK at PE partition offsets 0/64 (concurrent row
groups). Per-pair PSUM accumulators are double-buffered and the softmax
normalization is emitted interleaved into the next pair's loop.
Post-attention weights (Wo/W1/W2/x residual) prefetch during the gather.
"""

import numpy as np
import ml_dtypes

import concourse.bass as bass
import concourse.mybir as mybir
import concourse.tile as tile
from concourse import bacc
from concourse.bass_utils import run_bass_kernel_spmd

f32 = mybir.dt.float32
bf16 = mybir.dt.bfloat16

NCORES = 8
T = 4096
C = 768
F = 3072
NH = 12
D = 64
TL = T // NCORES          # 512 local tokens per core
CCH = C // 128            # 6
FCH = F // 128            # 24
NQC = TL // 128           # 4 query chunks of 128
NSLOT = 4                 # kv slots: 128 local columns each
NHP = NH // 2             # 6 head pairs
EPS = 1e-5
K_HP = 128 * TL           # 65536 elems: one head pair's K^T payload
V_HP = NQC * 128 * 130    # 66560 elems: one head pair's V (+ones) payload
K_ALL = NHP * K_HP        # 393216
V_ALL = NHP * V_HP        # 399360
KV_RANK = K_ALL + V_ALL   # combined per-rank K+V payload (one AllGather)
SCALE = 1.0 / np.sqrt(D)
ADD = mybir.AluOpType.add
MULT = mybir.AluOpType.mult
SUB = mybir.AluOpType.subtract


def _ap(handle, offset, pattern):
    return bass.AP(tensor=handle, offset=offset, ap=[list(p) for p in pattern])


class _Bacc(bacc.Bacc):
    """Bacc whose activation-table placement maps Exp and Ln exclusively to
    the natural_log_exp_and_others set, so the softmax exp, the exp(-ln(x))
    denominators and the LayerNorm rsqrt all share one resident table
    (the default first-match policy alternates natural_log / exp_and_others
    and reloads tables on every switch)."""

    def insert_act_table_loads(self):
        import bass_rust as _bass_rust
        from concourse.hw_specs import get_activation_tables
        has_activation = any(
            isinstance(i, mybir.InstActivation)
            for b in self.main_func.blocks
            for i in b.instructions
        )
        if not has_activation:
            return
        exp_ln = {mybir.ActivationFunctionType.Exp,
                  mybir.ActivationFunctionType.Ln}
        tables = []
        for name, fns in get_activation_tables(self.m.arch).items():
            if name != "natural_log_exp_and_others":
                fns = set(fns) - exp_ln
            tables.append((name, set(fns)))
        _bass_rust.insert_act_table_loads(self, tables)


def build_nc():
    nc = _Bacc("TRN2", target_bir_lowering=False, debug=False, num_devices=NCORES)

    # ---- I/O ----
    xT_in = nc.declare_dram_parameter("xT", [C, TL], bf16, isOutput=False)
    xtk_in = nc.declare_dram_parameter("xtk", [TL, C], f32, isOutput=False)
    mk_in = nc.declare_dram_parameter("masks", [128, NCORES * 128], bf16, isOutput=False)
    wq_in = nc.declare_dram_parameter("wq", [C, C], bf16, isOutput=False)
    wk_in = nc.declare_dram_parameter("wk", [C, C], bf16, isOutput=False)
    wv_in = nc.declare_dram_parameter("wv", [C, C], bf16, isOutput=False)
    wo_in = nc.declare_dram_parameter("wo", [C, C], bf16, isOutput=False)
    w1_in = nc.declare_dram_parameter("w1", [C, F], bf16, isOutput=False)
    w2_in = nc.declare_dram_parameter("w2", [F, C], bf16, isOutput=False)
    # packed per-partition (feature-major) biases: bq(6) bk(6) b1(24)
    colb_in = nc.declare_dram_parameter("colb", [128, 36], f32, isOutput=False)
    # packed broadcast-row consts: bo bv g1 h1 b2 g2 h2
    rowb_in = nc.declare_dram_parameter("rowb", [7, C], f32, isOutput=False)
    y_out = nc.declare_dram_parameter("y", [TL, C], f32, isOutput=True)

    # two gather chunks: head pairs 0-2 and 3-5 (K block then V block each)
    KV_CH = 3 * (K_HP + V_HP)
    kv_loc = [nc.dram_tensor(f"kv_loc{g}", [KV_CH], bf16) for g in range(2)]
    kv_gath = [nc.dram_tensor(f"kv_gath{g}", [NCORES * KV_CH], bf16,
                              addr_space="Shared") for g in range(2)]

    id_bf_d = nc.inline_tensor(np.eye(128).astype(ml_dtypes.bfloat16), name="id_bf_d")

    with tile.TileContext(nc) as tc:
        import contextlib
        with contextlib.ExitStack() as ctx:
            consts = ctx.enter_context(tc.tile_pool(name="consts", bufs=1))
            actp = ctx.enter_context(tc.tile_pool(name="actp", bufs=1))
            w1pool = ctx.enter_context(tc.tile_pool(name="w1pool", bufs=1))
            postp = ctx.enter_context(tc.tile_pool(name="postp", bufs=1))
            warmp = ctx.enter_context(tc.tile_pool(name="warmp", bufs=1))

            # ---- PE warm-up: flip HAM to K=8/8 while the first DMAs land ----
            warm_sb = warmp.tile([128, 512], bf16, name="warm_sb")
            nc.vector.memset(warm_sb, 0.01)

            def pe_burn(n, tag):
                with tc.tile_pool(name=f"warmps{tag}", bufs=1,
                                  space="PSUM") as wpool:
                    wps = wpool.tile([128, 512], f32, name=f"wps{tag}")
                    for _ in range(n):
                        nc.tensor.matmul(wps, lhsT=warm_sb[:, 0:128],
                                         rhs=warm_sb, start=True, stop=True)

            pe_burn(40, "a")

            eps_t = consts.tile([128, 1], f32, name="eps_t")
            nc.vector.memset(eps_t, EPS)
            colb = consts.tile([128, 36], f32, name="colb")
            nc.sync.dma_start(out=colb, in_=colb_in[:, :])
            bq_sb = colb[:, 0:CCH]
            bk_sb = colb[:, CCH:2 * CCH]
            b1_sb = colb[:, 2 * CCH:2 * CCH + FCH]

            # warm the ACT ln/exp table before attention needs it
            dume = consts.tile([1, 1], f32, name="dume")
            nc.scalar.activation(out=dume, in_=eps_t[0:1, 0:1],
                                 func=mybir.ActivationFunctionType.Ln, scale=1.0)
            nc.scalar.activation(out=dume, in_=dume,
                                 func=mybir.ActivationFunctionType.Exp, scale=1.0)

            aoT = [actp.tile([128, TL], bf16, name=f"aoT_{i}") for i in range(CCH)]
            bc = {}

            def bc_load(nm):
                j = ["bo", "bv", "g1", "h1", "b2", "g2", "h2"].index(nm)
                t = consts.tile([128, C], f32, name=f"bc_{nm}")
                nc.sync.dma_start(out=t, in_=_ap(rowb_in, j * C, [[0, 128], [1, C]]))
                bc[nm] = t

            with tc.tile_pool(name="qkvlive", bufs=1) as qkvlive:
                qt_b = [qkvlive.tile([128, TL], bf16, name=f"qt_{ch}")
                        for ch in range(CCH)]

                # ---- per-head-pair K/Q/V + one combined AllGather ----
                with tc.tile_pool(name="wproj", bufs=2) as wproj, \
                     tc.tile_pool(name="psumq", bufs=4, space="PSUM") as psum:
                    # critical-path loads first: x^T (bf16), then weights
                    xt_b = []
                    for ch in range(CCH):
                        tb = wproj.tile([128, TL], bf16, name=f"xt_b_{ch}")
                        nc.sync.dma_start(
                            out=tb, in_=xT_in[128 * ch:128 * (ch + 1), :])
                        xt_b.append(tb)

                    def load_ws(handle, pfx):
                        ts = []
                        for kch in range(CCH):
                            t = wproj.tile([128, C], bf16, name=f"{pfx}_{kch}")
                            nc.sync.dma_start(
                                out=t, in_=handle[128 * kch:128 * (kch + 1), :])
                            ts.append(t)
                        return ts

                    wk_t = load_ws(wk_in, "wk")
                    wv_t = load_ws(wv_in, "wv")
                    wq_t = load_ws(wq_in, "wq")
                    bc_load("bv")

                    # ---- K^T + V per gather chunk (hps 3g..3g+2), then that
                    # chunk's AllGather: attention on hp 0-2 overlaps the
                    # second chunk's gather ----
                    for g in range(2):
                        for hp in range(3 * g, 3 * g + 3):
                            lo = 128 * hp
                            ps = psum.tile([128, TL], f32, name="pp", tag="pp")
                            for kch in range(CCH):
                                nc.tensor.matmul(
                                    ps, lhsT=wk_t[kch][:, lo:lo + 128],
                                    rhs=xt_b[kch],
                                    start=(kch == 0), stop=(kch == CCH - 1))
                            kt = wproj.tile([128, TL], bf16, name=f"kt_{hp}",
                                            tag="kt_t", bufs=2)
                            nc.vector.tensor_scalar(
                                out=kt, in0=ps, scalar1=bk_sb[:, hp:hp + 1],
                                scalar2=None, op0=ADD)
                            nc.sync.dma_start(
                                out=_ap(kv_loc[g], (hp - 3 * g) * K_HP,
                                        [[TL, 128], [1, TL]]),
                                in_=kt)

                        for tch in range(NQC):
                            nh2 = g
                            ps = psum.tile([128, 384], f32, name="pv", tag="pv")
                            for kch in range(CCH):
                                nc.tensor.matmul(
                                    ps,
                                    lhsT=xt_b[kch][:, 128 * tch:128 * (tch + 1)],
                                    rhs=wv_t[kch][:, 384 * nh2:384 * (nh2 + 1)],
                                    start=(kch == 0), stop=(kch == CCH - 1))
                            vt = wproj.tile([128, 6, 65], bf16,
                                            name=f"v_{tch}_{nh2}", tag="v_t", bufs=3)
                            nc.vector.tensor_tensor(
                                out=vt[:, :, 0:D],
                                in0=ps.rearrange("p (h d) -> p h d", d=D),
                                in1=bc["bv"][:, 384 * nh2:384 * (nh2 + 1)].rearrange(
                                    "p (h d) -> p h d", d=D),
                                op=ADD)
                            nc.vector.memset(vt[:, :, D:D + 1], 1.0)
                            nc.sync.dma_start(
                                out=_ap(kv_loc[g],
                                        3 * K_HP + tch * 130,
                                        [[NQC * 130, 128], [V_HP, 3], [1, 130]]),
                                in_=vt)

                        nc.gpsimd.collective_compute(
                            "AllGather", mybir.AluOpType.bypass,
                            replica_groups=[list(range(NCORES))],
                            ins=[kv_loc[g][:]], outs=[kv_gath[g][:]])

                    # masks: needed at attention start, small
                    mk_sb = consts.tile([128, NCORES * 128], bf16, name="mk_sb")
                    nc.sync.dma_start(out=mk_sb, in_=mk_in[:, :])

                    # ---- Q^T (overlaps the collective) ----
                    for hp in range(NHP):
                        lo = 128 * hp
                        ps = psum.tile([128, TL], f32, name="pp", tag="pp")
                        for kch in range(CCH):
                            nc.tensor.matmul(
                                ps, lhsT=wq_t[kch][:, lo:lo + 128],
                                rhs=xt_b[kch], start=(kch == 0), stop=(kch == CCH - 1))
                        nc.vector.tensor_scalar(
                            out=qt_b[hp], in0=ps, scalar1=bq_sb[:, hp:hp + 1],
                            scalar2=None, op0=ADD)

                # ---- Wo / residual prefetch: issued while the gather runs so
                # the post-attention phases start with data resident ----
                for nm in ["bo", "g1", "h1", "b2", "g2", "h2"]:
                    bc_load(nm)
                id_bf = consts.tile([128, 128], bf16, name="id_bf")
                nc.sync.dma_start(out=id_bf, in_=id_bf_d[:])
                wo_t = []
                for kch in range(CCH):
                    t = w1pool.tile([128, C], bf16, name=f"wo_{kch}")
                    nc.sync.dma_start(out=t, in_=wo_in[128 * kch:128 * (kch + 1), :])
                    wo_t.append(t)
                x_tok = []
                for tch in range(NQC):
                    t = postp.tile([128, C], f32, name=f"xtok_{tch}")
                    nc.sync.dma_start(out=t, in_=xtk_in[128 * tch:128 * (tch + 1), :])
                    x_tok.append(t)
                w1_t = []

                # keep the PE array active across the gather window so
                # attention starts at full clock
                pe_burn(192, "b")

                # ---- attention ----
                with tc.tile_pool(name="kvstage", bufs=1) as kvstage, \
                     tc.tile_pool(name="atw", bufs=4) as atw, \
                     tc.tile_pool(name="atp", bufs=1, space="PSUM") as atp:
                    pending = []   # deferred normalize pieces from previous hp

                    def emit_pending(n):
                        for _ in range(min(n, len(pending))):
                            pending.pop(0)()

                    for hp in range(NHP):
                        if hp == NHP - 2:
                            # W1 prefetch: SBUF for it frees up once the early
                            # staging buffers retire; MLP needs it ~40us later
                            for kch in range(CCH):
                                t = w1pool.tile([128, F], bf16, name=f"w1_{kch}")
                                nc.sync.dma_start(
                                    out=t, in_=w1_in[128 * kch:128 * (kch + 1), :])
                                w1_t.append(t)
                        # per-rank staging so QK(r) starts as soon as rank r lands
                        g, hpl = hp // 3, hp % 3
                        ktg = kvstage.tile([128, NCORES, TL], bf16, name=f"ktg_{hp}",
                                           tag="ktg", bufs=2)
                        for r in range(NCORES):
                            nc.sync.dma_start(
                                out=ktg[:, r, :],
                                in_=_ap(kv_gath[g], r * KV_CH + hpl * K_HP,
                                        [[TL, 128], [1, TL]]))
                        vag = kvstage.tile([128, NCORES, NQC, 130], bf16,
                                           name=f"vag_{hp}", tag="vag", bufs=2)
                        for r in range(NCORES):
                            nc.sync.dma_start(
                                out=vag[:, r, :, :],
                                in_=_ap(kv_gath[g], r * KV_CH + 3 * K_HP
                                        + hpl * V_HP,
                                        [[NQC * 130, 128], [1, NQC * 130]]))

                        acc = atp.tile([65, 2, TL], f32, name="acc", tag="acc",
                                       bufs=2)
                        # A@V is emitted one tile behind QK/exp/mask so the PE
                        # FIFO never head-of-line blocks on the exp of the
                        # current tile (software pipelining)
                        av_q = []
                        it = 0
                        ntile = NSLOT * NCORES
                        for s in range(NSLOT):
                            q0 = 128 * s
                            nq = TL - q0
                            for r in range(NCORES):
                                lg = atp.tile([128, 2, TL], f32, name="lg",
                                              tag="lg", bufs=2)
                                for i in range(2):
                                    ho = 64 * i
                                    nc.tensor.matmul(
                                        lg[:, i, 0:nq],
                                        lhsT=ktg[ho:ho + 64, r,
                                                 128 * s:128 * (s + 1)],
                                        rhs=qt_b[hp][ho:ho + 64, q0:TL],
                                        start=True, stop=True)
                                pr = atw.tile([128, 2, TL], bf16, name="pr",
                                              tag="pr", bufs=8)
                                nc.scalar.activation(
                                    out=pr[:, :, 0:nq], in_=lg[:, :, 0:nq],
                                    func=mybir.ActivationFunctionType.Exp,
                                    scale=SCALE)
                                mb = mk_sb[:, 128 * r:128 * (r + 1)]
                                nc.vector.tensor_tensor(
                                    out=pr[:, :, 0:128], in0=pr[:, :, 0:128],
                                    in1=bass.AP(tensor=mb.tensor, offset=mb.offset,
                                                ap=[list(mb.ap[0]), [0, 2],
                                                    list(mb.ap[1])]),
                                    op=MULT)

                                def av(s=s, r=r, q0=q0, nq=nq, pr=pr, vag=vag,
                                       acc=acc, idx=it):
                                    for i in range(2):
                                        nc.tensor.matmul(
                                            acc[:, i, q0:TL],
                                            lhsT=vag[:, r, s,
                                                     65 * i:65 * (i + 1)],
                                            rhs=pr[:, i, 0:nq],
                                            start=(idx == 0),
                                            stop=(idx == ntile - 1))
                                av_q.append(av)
                                if len(av_q) > 1:
                                    av_q.pop(0)()
                                it += 1
                                if it % 3 == 0:
                                    emit_pending(1)
                        while av_q:
                            av_q.pop(0)()

                        # deferred normalize: reciprocal of the denominators on
                        # the Scalar engine as exp(-ln(x)) (same table set as
                        # the attention exp), then partition broadcast +
                        # multiply, interleaved into the next head pair's loop
                        def make_norm(hp, acc):
                            lnd = atw.tile([1, 2, TL], f32, name="lnd", tag="lnd",
                                           bufs=2)
                            rec = atw.tile([1, 2, TL], f32, name="rec", tag="rec",
                                           bufs=2)
                            pieces = []

                            def p_ln(lnd=lnd, acc=acc):
                                nc.scalar.activation(
                                    out=lnd, in_=acc[D:D + 1, :, :],
                                    func=mybir.ActivationFunctionType.Ln,
                                    scale=1.0)

                            def p_exp(rec=rec, lnd=lnd):
                                nc.scalar.activation(
                                    out=rec, in_=lnd,
                                    func=mybir.ActivationFunctionType.Exp,
                                    scale=-1.0)

                            pieces.append(p_ln)
                            pieces.append(p_exp)
                            for i in range(2):
                                def norm(i=i, rec=rec, acc=acc, hp=hp):
                                    brd = atw.tile([64, TL], f32, name="brd",
                                                   tag=f"brd{i}", bufs=2)
                                    nc.gpsimd.partition_broadcast(brd, rec[:, i, :])
                                    nc.vector.tensor_tensor(
                                        out=aoT[hp][64 * i:64 * (i + 1), :],
                                        in0=acc[0:D, i, :], in1=brd, op=MULT)
                                pieces.append(norm)
                            return pieces

                        emit_pending(99)
                        pending = make_norm(hp, acc)
                    emit_pending(99)

            # ---- W2 prefetch now that the KV staging SBUF is free ----
            w2pool = ctx.enter_context(tc.tile_pool(name="w2pool", bufs=1))
            w2_t = []
            for kch in range(FCH):
                t = w2pool.tile([128, C], bf16, name=f"w2t_{kch}")
                nc.sync.dma_start(out=t, in_=w2_in[128 * kch:128 * (kch + 1), :])
                w2_t.append(t)

            h_tok = [postp.tile([128, C], bf16, name=f"htok_{t}") for t in range(NQC)]
            hT = [postp.tile([128, TL], bf16, name=f"hT_{i}") for i in range(CCH)]
            mup = [postp.tile([128, TL], bf16, name=f"mup_{i}") for i in range(FCH)]
            lnw = ctx.enter_context(tc.tile_pool(name="lnw", bufs=2))

            def layernorm_apply(r_tile, g_bc, h_bc, out_tile, tag):
                st = lnw.tile([128, 3, 6], f32, name=f"st{tag}", tag=f"st{tag}")
                for sg in range(3):
                    nc.vector.bn_stats(
                        out=st[:, sg, :], in_=r_tile[:, 256 * sg:256 * (sg + 1)])
                mv = lnw.tile([128, 2], f32, name=f"mv{tag}", tag=f"mv{tag}")
                nc.vector.bn_aggr(out=mv, in_=st)
                # rsqrt(var + eps) = exp(-0.5 * ln(var + eps)): stays in the
                # natural_log_exp table set (no sqrt table switch)
                lnv = lnw.tile([128, 1], f32, name=f"lnv{tag}", tag=f"lnv{tag}")
                nc.scalar.activation(
                    out=lnv, in_=mv[:, 1:2],
                    func=mybir.ActivationFunctionType.Ln, bias=eps_t, scale=1.0)
                rs = lnw.tile([128, 1], f32, name=f"rs{tag}", tag=f"rs{tag}")
                nc.scalar.activation(
                    out=rs, in_=lnv,
                    func=mybir.ActivationFunctionType.Exp, scale=-0.5)
                tn = lnw.tile([128, C], f32, name=f"tn{tag}", tag=f"tn{tag}")
                nc.vector.tensor_scalar(
                    out=tn, in0=r_tile, scalar1=mv[:, 0:1], scalar2=rs,
                    op0=SUB, op1=MULT)
                nc.vector.tensor_tensor(out=tn, in0=tn, in1=g_bc, op=MULT)
                nc.vector.tensor_tensor(out=out_tile, in0=tn, in1=h_bc, op=ADD)

            # ---- Wo projection token-major + residual + LN1 + h transpose ----
            with tc.tile_pool(name="wop", bufs=1, space="PSUM") as wop:
                # h transposes are emitted one token chunk behind Wo+LN1 so
                # the PE isn't head-of-line blocked on the LN chain
                tr_q = []
                for tch in range(NQC):
                    psA = wop.tile([128, 384], f32, name="woA", tag="woA", bufs=2)
                    psB = wop.tile([128, 384], f32, name="woB", tag="woB", bufs=2)
                    for kch in range(CCH):
                        lt = aoT[kch][:, 128 * tch:128 * (tch + 1)]
                        nc.tensor.matmul(psA, lhsT=lt, rhs=wo_t[kch][:, 0:384],
                                         start=(kch == 0), stop=(kch == CCH - 1))
                        nc.tensor.matmul(psB, lhsT=lt, rhs=wo_t[kch][:, 384:C],
                                         start=(kch == 0), stop=(kch == CCH - 1))
                    r1 = lnw.tile([128, C], f32, name="r1", tag="r1")
                    nc.vector.tensor_tensor(out=r1[:, 0:384], in0=psA,
                                            in1=bc["bo"][:, 0:384], op=ADD)
                    nc.vector.tensor_tensor(out=r1[:, 384:C], in0=psB,
                                            in1=bc["bo"][:, 384:C], op=ADD)
                    nc.vector.tensor_tensor(out=r1, in0=r1, in1=x_tok[tch], op=ADD)
                    layernorm_apply(r1, bc["g1"], bc["h1"], h_tok[tch], "1")

                    def tr(tch=tch):
                        for ch in range(CCH):
                            tp = wop.tile([128, 128], bf16, name="tp", tag="tp",
                                          bufs=2)
                            nc.tensor.transpose(
                                tp, in_=h_tok[tch][:, 128 * ch:128 * (ch + 1)],
                                identity=id_bf)
                            nc.scalar.copy(
                                out=hT[ch][:, 128 * tch:128 * (tch + 1)], in_=tp)
                    tr_q.append(tr)
                    if len(tr_q) > 1:
                        tr_q.pop(0)()
                while tr_q:
                    tr_q.pop(0)()

            # ---- fused W1+gelu and W2 (token-major), pipelined over F-chunks ----
            # PSUM: pp1 (2 banks) + psC/psD for two token chunks (4 banks) = 6.
            # Token chunks are processed in two passes; W1 runs in the first.
            with tc.tile_pool(name="mlpp", bufs=1, space="PSUM") as mlpp:

                def w2_pass(tchs, with_w1):
                    psC = {t: mlpp.tile([128, 384], f32, name=f"w2A{t}",
                                        tag=f"w2A_{t % 2}", bufs=1) for t in tchs}
                    psD = {t: mlpp.tile([128, 384], f32, name=f"w2B{t}",
                                        tag=f"w2B_{t % 2}", bufs=1) for t in tchs}

                    def w2_chunk(m):
                        for tch in tchs:
                            lt = mup[m][:, 128 * tch:128 * (tch + 1)]
                            nc.tensor.matmul(
                                psC[tch], lhsT=lt, rhs=w2_t[m][:, 0:384],
                                start=(m == 0), stop=(m == FCH - 1))
                            nc.tensor.matmul(
                                psD[tch], lhsT=lt, rhs=w2_t[m][:, 384:C],
                                start=(m == 0), stop=(m == FCH - 1))

                    if with_w1:
                        for m in range(FCH):
                            ps = mlpp.tile([128, TL], f32, name="pp1", tag="pp1",
                                           bufs=2)
                            for kch in range(CCH):
                                nc.tensor.matmul(
                                    ps, lhsT=w1_t[kch][:, 128 * m:128 * (m + 1)],
                                    rhs=hT[kch],
                                    start=(kch == 0), stop=(kch == CCH - 1))
                            nc.scalar.activation(
                                out=mup[m], in_=ps,
                                func=mybir.ActivationFunctionType.Gelu,
                                bias=b1_sb[:, m:m + 1], scale=1.0)
                            if m > 0:
                                w2_chunk(m - 1)
                        w2_chunk(FCH - 1)
                    else:
                        for m in range(FCH):
                            w2_chunk(m)

                    # residual + LN2 -> y for these token chunks
                    for tch in tchs:
                        r2 = lnw.tile([128, C], f32, name="r2", tag="r2")
                        nc.vector.tensor_tensor(out=r2[:, 0:384], in0=psC[tch],
                                                in1=bc["b2"][:, 0:384], op=ADD)
                        nc.vector.tensor_tensor(out=r2[:, 384:C], in0=psD[tch],
                                                in1=bc["b2"][:, 384:C], op=ADD)
                        nc.vector.tensor_tensor(out=r2, in0=r2, in1=h_tok[tch],
                                                op=ADD)
                        yt = lnw.tile([128, C], f32, name="yt", tag="yt")
                        layernorm_apply(r2, bc["g2"], bc["h2"], yt, "2")
                        nc.sync.dma_start(
                            out=y_out[128 * tch:128 * (tch + 1), :], in_=yt)

                w2_pass([0, 1], True)
                w2_pass([2, 3], False)

    nc.compile()
    return nc


_NC_CACHE = None


def _get_nc():
    global _NC_CACHE
    if _NC_CACHE is None:
        _NC_CACHE = build_nc()
    return _NC_CACHE


def make_in_maps(inputs):
    x = np.asarray(inputs["x"], dtype=np.float32)      # [1, T, C]
    to_bf = lambda a: np.asarray(a, dtype=np.float32).astype(ml_dtypes.bfloat16)
    f32a = lambda k: np.asarray(inputs[k], np.float32)
    colb = np.concatenate([
        f32a("bq").reshape(CCH, 128).T,
        f32a("bk").reshape(CCH, 128).T,
        f32a("b1").reshape(FCH, 128).T,
    ], axis=1)                                          # [128, 36]
    rowb = np.stack([f32a("bo"), f32a("bv"), f32a("ln1_g"), f32a("ln1_b"),
                     f32a("b2"), f32a("ln2_g"), f32a("ln2_b")])  # [7, 768]
    shared = {
        "wq": to_bf(inputs["Wq"]), "wk": to_bf(inputs["Wk"]),
        "wv": to_bf(inputs["Wv"]), "wo": to_bf(inputs["Wo"]),
        "w1": to_bf(inputs["W1"]), "w2": to_bf(inputs["W2"]),
        "colb": np.ascontiguousarray(colb), "rowb": np.ascontiguousarray(rowb),
    }
    ki = np.arange(128)[:, None]
    qi = np.arange(128)[None, :]
    in_maps = []
    for c in range(NCORES):
        xs = x[0, c::NCORES, :]                                 # [TL, C]
        # multiplicative 0/1 causal masks for the diagonal kv slot, packed
        masks = np.concatenate([
            (NCORES * ki + r <= NCORES * qi + c) for r in range(NCORES)
        ], axis=1).astype(ml_dtypes.bfloat16)                   # [128, 8*128]
        m = dict(shared)
        m["xT"] = np.ascontiguousarray(xs.T).astype(ml_dtypes.bfloat16)
        m["xtk"] = np.ascontiguousarray(xs)
        m["masks"] = np.ascontiguousarray(masks)
        in_maps.append(m)
    return in_maps


def kernel(**inputs):
    nc = _get_nc()
    in_maps = make_in_maps(inputs)
    res = run_bass_kernel_spmd(nc, in_maps, list(range(NCORES)))
    x = np.asarray(inputs["x"])
    out = np.empty((1, T, C), dtype=np.float32)
    for c in range(NCORES):
        out[0, c::NCORES, :] = res.results[c]["y"]
    return out.astype(x.dtype) if x.dtype != np.float32 else out
